# revision 20
# baseline (speedup 1.0000x reference)
"""Causal multi-head attention (B=2, S=2048, D=1024, H=16, DH=64) on 8 TRN2 cores.

Sharding: core c handles batch b = c//4 and head group g = c%4 (4 heads, 256
feature cols).  Each core computes Q/K/V projections for its heads, causal
attention, and a partial output projection; the host sums the 4 partials per
batch.

Device program (per core), all SBUF operands bf16 (PSUM accum stays f32):
  - host pre-transposes X[b] -> XT [D, S] and weight slices, casting to bf16
    (bf16 keeps matmuls at 1 cycle/col at any width; fp8 would breach the
    2e-2 error gate since quantization noise does not average down in
    random-sign contractions).
  - Q^T, K^T kept as [o, s] (2 heads per 128-partition tile); PV stationary
    blocks per (s-chunk, pair): head A = [V | ones] (65 cols; acc rows 0:63
    = out^T, row 64 = denominator), head B = [ones | 0 | V] (128 cols; acc
    row 0 = denominator, rows 64:127 = out^T) so head B's output lands at
    partitions 64-127 directly -- no partition-shift DMA.
  - logits computed transposed (L^T = K_h Q_h^T) into PSUM f32; exp on ACT
    writes bf16 pt tiles; causal triangle masking via gpsimd affine_select
    in place (only the [128,128] diagonal sub-blocks, on the otherwise-idle
    Pool engine); PV accumulates in PSUM f32.
  - causal fine-grain: for diagonal k-chunks only q-cols >= 128j-512t are
    computed in logits/exp/PV.
  - normalization: 1/denominator = exp(-ln d) on ACT (same exp table set),
    NOT DVE reciprocal -- the DVE iterative divide measured ~2-3us per
    [1,512] on HW and head-of-line-blocks the projection copies behind it
    (counterfactual timing: -36us).  ACT ln of each denominator row, PE
    outer-product broadcast of ln d to both partition halves, one ACT
    exp(scale=-1) -> bcast, two DVE muls write outTn in place.  Pair-1
    norms deferred into the next tile's stream (carry).
  - emission interleave: a minimal projection prologue (K/Q pair 0 window 0,
    V s-chunks 0-3), then ALL remaining projection + output-projection work
    is chopped into ~1-2us PE "filler" units drained between each attention
    jg's logits and PV, so PE always has dep-ready work while ACT runs the
    exp stream (phase-alternation costed ~60us of engine idling on HW).
  - PSUM budget (8 banks) caps pipeline depth: lt [128,1024]x2 (4 banks) +
    shared 1-bank pool x4 (proj/out-proj outs, PV accs); norm bcast uses an
    lt slot.

Measured (8 axon-tunneled TRN2 cores): rel err 3.59e-03 vs f32 reference;
~215.5 us/exec by 1-vs-101 hardware-loop slope timing (session-start
baseline 245.4 us measured the same way; CoreSim cost model ~150 us --
the model misses DVE-divide cost, cross-engine sem latency, and HAM).
"""

import os
import numpy as np

# timing-experiment switches (comma-separated): "norecip", "nomask"
_KVAR = set(filter(None, os.environ.get("KVAR", "").split(",")))

B, S, D = 2, 2048, 1024
H, DH = 16, 64
NCORES = 8
GROUPS = 4          # head groups (one per core within a batch)
HPC = H // GROUPS   # heads per core = 4
O = HPC * DH        # per-core feature cols = 256
DC = D // 128       # contraction chunks = 8
NQT = S // 512      # q tiles = 4
NST = S // 128      # s chunks = 16

_PROGRAM = None
LAST_RESULTS = None  # stashed BassKernelResults for test harness introspection


def _build_program(loop_n=1):
    import concourse.bass as bass
    import concourse.tile as tile
    from concourse import bacc, mybir
    from contextlib import ExitStack

    f32 = mybir.dt.float32
    bf16 = mybir.dt.bfloat16
    ts = bass.ts
    Exp = mybir.ActivationFunctionType.Exp
    Ln = mybir.ActivationFunctionType.Ln
    Copy = mybir.ActivationFunctionType.Copy

    nc = bacc.Bacc("TRN2", target_bir_lowering=False, debug=False,
                   num_devices=NCORES)

    xt = nc.dram_tensor("xt", [D, S], bf16, kind="ExternalInput").ap()
    wqt = nc.dram_tensor("wqt", [D, O], bf16, kind="ExternalInput").ap()
    wkt = nc.dram_tensor("wkt", [D, O], bf16, kind="ExternalInput").ap()
    wvt = nc.dram_tensor("wvt", [D, O], bf16, kind="ExternalInput").ap()
    wot = nc.dram_tensor("wot", [O, D], bf16, kind="ExternalInput").ap()
    y = nc.dram_tensor("y", [S, D], f32, kind="ExternalOutput").ap()

    with tile.TileContext(nc) as tc, ExitStack() as ctx:
        per = ctx.enter_context(tc.tile_pool(name="per", bufs=1))
        xtp = ctx.enter_context(tc.tile_pool(name="xtp", bufs=2))
        work = ctx.enter_context(tc.tile_pool(name="work", bufs=6))
        ps_sm = ctx.enter_context(tc.tile_pool(name="ps_sm", bufs=4, space="PSUM"))
        ps_lt = ctx.enter_context(tc.tile_pool(name="ps_lt", bufs=2, space="PSUM"))

        # ---- persistent tiles -------------------------------------------------
        wq_sb = per.tile([128, DC, O], bf16, tag="wq")
        wk_sb = per.tile([128, DC, O], bf16, tag="wk")
        wv_sb = per.tile([128, DC, O], bf16, tag="wv")
        wo_sb = per.tile([128, 2, D], bf16, tag="wo")
        qT = per.tile([128, 2, S], bf16, tag="qT")
        kT = per.tile([128, 2, S], bf16, tag="kT")
        # PV stationary blocks, one per (s-chunk, pair, h01), each 129 cols:
        #   h01=0 (head A): V at cols 0:64, ones at col 64
        #     -> acc rows 0:63 = out_A^T, row 64 = denom_A
        #   h01=1 (head B): ones at col 0, zeros 1:64, V at cols 64:128
        #     -> acc row 0 = denom_B, rows 64:127 = out_B^T (no partition
        #        shift needed; outTn written in place by the norm muls)
        vones = per.tile([128, NST, 2, 2, DH * 2 + 1], bf16, tag="vones")
        outTn = per.tile([128, 2, S], bf16, tag="outTn")

        SH = S // 2

        # ones rows for the outer-product broadcast of softmax denominators:
        # row 64 (head A, out rows 0:63), row 0 (head B, out rows 64:127).
        # f32 because the broadcast rhs (ln of the denominator) is f32.
        ones_bc = per.tile([128, DH], f32, tag="ones_bc")
        nc.vector.memset(ones_bc[DH:DH + 1, :], 1.0)
        nc.vector.memset(ones_bc[0:1, :], 1.0)
        for st in range(NST):
            nc.vector.memset(vones[:, st, :, 0, DH:DH + 1], 1.0)
            nc.vector.memset(vones[:, st, :, 1, 0:DH], 0.0)
            nc.vector.memset(vones[:, st, :, 1, 0:1], 1.0)
        # dummy exp so the ACT exp-table load lands during the input-DMA
        # phase instead of stalling the first attention tile
        warm = per.tile([1, 8], bf16, tag="warm")
        nc.scalar.activation(warm[0:1, :], ones_bc[0:1, 0:8], Exp)

        # ---- phase 0 helper: input DMAs, chunked so the first projection
        # matmuls only wait on their own contraction chunk ---------------------
        def load_inputs():
            wkr = wkt.rearrange("(c p) o -> p c o", p=128)
            wqr = wqt.rearrange("(c p) o -> p c o", p=128)
            wvr = wvt.rearrange("(c p) o -> p c o", p=128)
            xt_sbs = [xtp.tile([128, DC, SH], bf16, tag="xt", name="xt")
                      for _ in range(2)]
            for dc in range(DC):
                nc.sync.dma_start(wk_sb[:, dc, :], wkr[:, dc, :])
                nc.sync.dma_start(
                    xt_sbs[0][:, dc, :], xt[dc * 128:(dc + 1) * 128, 0:SH])
            for dc in range(DC):
                nc.sync.dma_start(wq_sb[:, dc, :], wqr[:, dc, :])
                nc.sync.dma_start(wv_sb[:, dc, :], wvr[:, dc, :])
            for dc in range(DC):
                nc.sync.dma_start(
                    xt_sbs[1][:, dc, :], xt[dc * 128:(dc + 1) * 128, SH:S])
            nc.sync.dma_start(wo_sb[:], wot.rearrange("(c p) m -> p c m", p=128))
            return xt_sbs

        # ---- PE filler units (~0.85-1.7us each): projections and output
        # projection chunks, interleaved between attention jg iterations so
        # PE always has dep-ready work while the ACT exp stream runs ----------
        def kq_unit(xt_sbs, sh, w_sb, dst, pt_i, qt):
            def run():
                xt_sb = xt_sbs[sh]
                ps = ps_sm.tile([128, 512], f32, tag="sm", name="mm")
                for dc in range(DC):
                    nc.tensor.matmul(
                        ps[:],
                        w_sb[:, dc, ts(pt_i, 128)],
                        xt_sb[:, dc, ts(qt, 512)],
                        start=(dc == 0), stop=(dc == DC - 1),
                    )
                # psum f32 -> bf16 persistent on DVE (ACT is the attention
                # bottleneck engine; DVE has slack)
                nc.vector.tensor_copy(
                    dst[:, pt_i, ts(sh * 2 + qt, 512)], ps[:])
            return run

        def v_unit(xt_sbs, sh, st_l):
            def run():
                xt_sb = xt_sbs[sh]
                st = sh * (SH // 128) + st_l
                ps = ps_sm.tile([128, O], f32, tag="sm", name="mm")
                for dc in range(DC):
                    nc.tensor.matmul(
                        ps[:],
                        xt_sb[:, dc, ts(st_l, 128)],
                        wv_sb[:, dc, :],
                        start=(dc == 0), stop=(dc == DC - 1),
                    )
                # ps cols are (pair, h01, d); A-heads to block 0 cols 0:64,
                # B-heads to block 1 cols 64:128
                psr = ps[:].rearrange("p (pr hd d) -> p pr hd d", pr=2, hd=2)
                nc.vector.tensor_copy(
                    vones[:, st, :, 0, 0:DH], psr[:, :, 0, :])
                nc.vector.tensor_copy(
                    vones[:, st, :, 1, DH:2 * DH], psr[:, :, 1, :])
            return run

        def op_unit(st):
            def run():
                ys = work.tile([128, 1024], f32, tag="ystage")
                for mt in range(2):
                    ps = ps_sm.tile([128, 512], f32, tag="sm", name="mm")
                    for pair in range(2):
                        nc.tensor.matmul(
                            ps[:],
                            outTn[:, pair, ts(st, 128)],
                            wo_sb[:, pair, ts(mt, 512)],
                            start=(pair == 0), stop=(pair == 1),
                        )
                    nc.vector.tensor_copy(ys[:, ts(mt, 512)], ps[:])
                nc.sync.dma_start(y[ts(st, 128), :], ys[:])
            return run

        # ---- attention for one q tile; `fillers` (projection / out-proj
        # units) are drained between each jg's logits and PV so PE stays busy
        # while that jg's exps run on ACT -------------------------------------
        def attn_tile(t, fillers=(), carry_in=None):
            fillers = list(fillers)
            nfill = len(fillers)
            total_slots = 2 * (2 * t + 2)
            slot = drained = 0

            def drain():
                nonlocal slot, drained
                slot += 1
                while drained < nfill * slot // total_slots:
                    fillers[drained]()
                    drained += 1

            carry = carry_in
            for pair in range(2):
                accs = [ps_sm.tile([DH + 1, 512], f32, tag="sm", name="acc"),
                        ps_sm.tile([128, 512], f32, tag="sm", name="acc")]
                njp = 2 * t + 2  # j-pairs covering k chunks 0..4t+3
                for jg in range(njp):
                    if jg == 1 and pair == 0 and carry is not None:
                        carry()  # deferred previous-pair normalization
                        carry = None
                    j0, j1 = 2 * jg, 2 * jg + 1
                    # fine-grain causal: for diagonal k-chunks only q-cols
                    # >= 128j - 512t participate
                    c0 = max(0, 128 * j0 - 512 * t)
                    c1 = max(0, 128 * j1 - 512 * t)
                    lts = [ps_lt.tile([128, 1024], f32, tag="lt", name="lt")
                           for _ in range(2)]
                    # j1's logits start at c0 too (extra strip fully
                    # masked below) so exp runs as ONE ACT instruction
                    for j_half, j, c in ((0, j0, c0), (1, j1, c0)):
                        for h01 in range(2):
                            bp = 64 * h01
                            nc.tensor.matmul(
                                lts[h01][:, j_half * 512 + c:(j_half + 1) * 512],
                                kT[bp:bp + 64, pair, ts(j, 128)],
                                qT[bp:bp + 64, pair, 512 * t + c:512 * (t + 1)],
                                start=True, stop=True,
                            )
                    drain()  # PE filler between this jg's logits and its PV
                    for h01 in range(2):
                        pt = work.tile([128, 1024], bf16, tag="pt", bufs=6)
                        if "halfexp" in _KVAR:  # timing counterfactual
                            nc.scalar.activation(pt[:, 0:512], lts[h01][:, 0:512],
                                                 Exp, scale=DH ** -0.5)
                        elif c0 == 0:
                            nc.scalar.activation(pt[:], lts[h01][:], Exp,
                                                 scale=DH ** -0.5)
                        else:
                            nc.scalar.activation(
                                pt[:, c0:512], lts[h01][:, c0:512], Exp,
                                scale=DH ** -0.5)
                            nc.scalar.activation(
                                pt[:, 512 + c0:1024], lts[h01][:, 512 + c0:1024],
                                Exp, scale=DH ** -0.5)
                        # zero the strictly-upper triangle of diagonal blocks
                        # (j1 region widened to cover the masked [c0,c1) strip)
                        for j_half, j, c in ((0, j0, c0), (1, j1, c1)):
                            if "nomask" in _KVAR:  # timing counterfactual
                                continue
                            if j >= 4 * t:
                                lo = j_half * 512 + (c0 if j_half else c)
                                hi = j_half * 512 + c + 128
                                nc.gpsimd.affine_select(
                                    out=pt[:, lo:hi],
                                    in_=pt[:, lo:hi],
                                    compare_op=mybir.AluOpType.is_ge,
                                    fill=0.0,
                                    base=lo - (j_half * 512 + c),
                                    pattern=[[1, hi - lo]],
                                    channel_multiplier=-1,
                                )
                        lhsT = (vones[:, :, pair, 0, 0:DH + 1] if h01 == 0
                                else vones[:, :, pair, 1, 0:2 * DH])
                        for j_half, j, c in ((0, j0, c0), (1, j1, c1)):
                            nc.tensor.matmul(
                                accs[h01][:, c:512],
                                lhsT[:, j, :],
                                pt[:, j_half * 512 + c:(j_half + 1) * 512],
                                start=(jg == 0 and j_half == 0),
                                stop=(jg == njp - 1 and j_half == 1),
                            )
                def norm(accs=accs, pair=pair):
                    # softmax 1/denominator WITHOUT the DVE iterative-divide
                    # reciprocal (measured ~2-3us per [1,512] on HW): ACT ln
                    # of each denominator row (A: row 64 of acc0, B: row 0 of
                    # acc1), PE outer-product broadcast of ln(d) to the two
                    # partition halves, then one ACT exp(-x) writes the
                    # broadcast reciprocal; two DVE muls write outTn in place
                    lnd = work.tile([128, 512], f32, tag="recip")
                    nc.scalar.activation(lnd[DH:DH + 1, :],
                                         accs[0][DH:DH + 1, :], Ln)
                    nc.scalar.activation(lnd[0:1, :],
                                         accs[1][0:1, :], Ln)
                    bc_ps = ps_lt.tile([128, 1024], f32, tag="lt", name="bcps")
                    nc.tensor.matmul(bc_ps[0:DH, 0:512],
                                     ones_bc[DH:DH + 1, :],
                                     lnd[DH:DH + 1, :],
                                     start=True, stop=True)
                    nc.tensor.matmul(bc_ps[DH:128, 0:512],
                                     ones_bc[0:1, :],
                                     lnd[0:1, :],
                                     start=True, stop=True)
                    bcast = work.tile([128, 512], bf16, tag="bcast")
                    with nc.allow_low_precision(
                            reason="softmax reciprocal feeds bf16 pipeline"):
                        nc.scalar.activation(bcast[:, :], bc_ps[:, 0:512],
                                             Exp, scale=-1.0)
                    nc.vector.tensor_mul(outTn[0:DH, pair, ts(t, 512)],
                                         accs[0][0:DH, :], bcast[0:DH, :])
                    nc.vector.tensor_mul(outTn[DH:128, pair, ts(t, 512)],
                                         accs[1][DH:128, :], bcast[DH:128, :])
                if pair == 0:
                    norm()
                else:
                    carry = norm
            while drained < nfill:  # leftovers (shouldn't normally trigger)
                fillers[drained]()
                drained += 1
            return carry

        # ---- emission: a short projection prologue covering only attention
        # tile 0's needs, then attention tiles with all remaining projection
        # and output-projection work drained as per-jg PE fillers -------------
        def body():
            xt_sbs = load_inputs()
            # prologue: K/Q head-pair 0 for q-window 0, V s-chunks 0-3
            kq_unit(xt_sbs, 0, wk_sb, kT, 0, 0)()
            kq_unit(xt_sbs, 0, wq_sb, qT, 0, 0)()
            for sl in range(4):
                v_unit(xt_sbs, 0, sl)()
            c = attn_tile(0, fillers=[
                kq_unit(xt_sbs, 0, wk_sb, kT, 1, 0),
                kq_unit(xt_sbs, 0, wq_sb, qT, 1, 0),
                kq_unit(xt_sbs, 0, wk_sb, kT, 0, 1),
                kq_unit(xt_sbs, 0, wk_sb, kT, 1, 1),
                kq_unit(xt_sbs, 0, wq_sb, qT, 0, 1),
                kq_unit(xt_sbs, 0, wq_sb, qT, 1, 1),
                v_unit(xt_sbs, 0, 4),
                v_unit(xt_sbs, 0, 5),
            ])
            c = attn_tile(1, fillers=[
                v_unit(xt_sbs, 0, 6),
                v_unit(xt_sbs, 0, 7),
                kq_unit(xt_sbs, 1, wk_sb, kT, 0, 0),
                kq_unit(xt_sbs, 1, wk_sb, kT, 1, 0),
                kq_unit(xt_sbs, 1, wq_sb, qT, 0, 0),
                kq_unit(xt_sbs, 1, wq_sb, qT, 1, 0),
                op_unit(0), op_unit(1), op_unit(2), op_unit(3),
            ], carry_in=c)
            c = attn_tile(2, fillers=[
                v_unit(xt_sbs, 1, 0),
                v_unit(xt_sbs, 1, 1),
                v_unit(xt_sbs, 1, 2),
                v_unit(xt_sbs, 1, 3),
                v_unit(xt_sbs, 1, 4),
                v_unit(xt_sbs, 1, 5),
                kq_unit(xt_sbs, 1, wk_sb, kT, 0, 1),
                kq_unit(xt_sbs, 1, wk_sb, kT, 1, 1),
                kq_unit(xt_sbs, 1, wq_sb, qT, 0, 1),
                kq_unit(xt_sbs, 1, wq_sb, qT, 1, 1),
                op_unit(4), op_unit(5), op_unit(6), op_unit(7),
            ], carry_in=c)
            c = attn_tile(3, fillers=[
                v_unit(xt_sbs, 1, 6),
                v_unit(xt_sbs, 1, 7),
                op_unit(8), op_unit(9), op_unit(10), op_unit(11),
            ], carry_in=c)
            c()
            for st in range(12, 16):
                op_unit(st)()

        if loop_n == 1:
            body()
        else:
            with tc.For_i(0, loop_n, 1):
                body()

    nc.compile()
    return nc


def _get_program(loop_n=1):
    global _PROGRAM
    if _PROGRAM is None:
        _PROGRAM = {}
    if loop_n not in _PROGRAM:
        _PROGRAM[loop_n] = _build_program(loop_n)
    return _PROGRAM[loop_n]


def kernel(X, Wq, Wk, Wv, Wo):
    global LAST_RESULTS
    from concourse.bass_utils import run_bass_kernel_spmd

    X = np.asarray(X, dtype=np.float32)
    Wq = np.asarray(Wq, dtype=np.float32)
    Wk = np.asarray(Wk, dtype=np.float32)
    Wv = np.asarray(Wv, dtype=np.float32)
    Wo = np.asarray(Wo, dtype=np.float32)

    nc = _get_program()
    in_maps = _make_in_maps(X, Wq, Wk, Wv, Wo)
    res = run_bass_kernel_spmd(nc, in_maps, list(range(NCORES)))
    LAST_RESULTS = res

    out = np.empty((B, S, D), dtype=np.float32)
    for b in range(B):
        acc = res.results[b * GROUPS]["y"].astype(np.float32)
        for g in range(1, GROUPS):
            acc = acc + res.results[b * GROUPS + g]["y"]
        out[b] = acc
    return out


def _make_in_maps(X, Wq, Wk, Wv, Wo):
    import ml_dtypes
    bf16 = ml_dtypes.bfloat16
    xts = [np.ascontiguousarray(X[b].T).astype(bf16) for b in range(B)]
    in_maps = []
    for c in range(NCORES):
        b, g = divmod(c, GROUPS)
        rows = slice(g * O, (g + 1) * O)
        in_maps.append({
            "xt": xts[b],
            "wqt": np.ascontiguousarray(Wq[rows, :].T).astype(bf16),
            "wkt": np.ascontiguousarray(Wk[rows, :].T).astype(bf16),
            "wvt": np.ascontiguousarray(Wv[rows, :].T).astype(bf16),
            "wot": np.ascontiguousarray(Wo[:, rows].T).astype(bf16),
        })
    return in_maps


def build_timed_callable(in_maps=None, loop_n=1):
    """Build the same sharded jit callable bass2jax uses, with inputs
    pre-placed on the 8 devices, for repeat-timing the NEFF execution.

    With loop_n=K the device program wraps the whole kernel body (including
    input DMAs) in a K-iteration hardware loop, so per-exec device time can
    be measured as a slope between two loop counts, cancelling the (large,
    noisy) axon dispatch overhead."""
    import jax
    import numpy as np
    from jax.sharding import Mesh, PartitionSpec, NamedSharding
    from jax.experimental.shard_map import shard_map
    from concourse import bass2jax, mybir

    nc = _get_program(loop_n)
    bass2jax.install_neuronx_cc_hook()

    if in_maps is None:
        import test as _t
        inputs, _ = _t.get_reference_data()
        in_maps = _make_in_maps(**inputs)

    partition_name = (
        nc.partition_id_tensor.name if nc.partition_id_tensor else None)
    in_names, out_names, out_avals, zero_shapes = [], [], [], []
    for alloc in nc.m.functions[0].allocations:
        if not isinstance(alloc, mybir.MemoryLocationSet):
            continue
        name = alloc.memorylocations[0].name
        if alloc.kind == "ExternalInput":
            if name != partition_name:
                in_names.append(name)
        elif alloc.kind == "ExternalOutput":
            out_names.append(name)
            shape = tuple(alloc.tensor_shape)
            out_avals.append(
                jax.core.ShapedArray(shape, mybir.dt.np(alloc.dtype)))
            zero_shapes.append((NCORES * shape[0], *shape[1:]))
    n_params = len(in_names)
    n_out = len(out_names)
    # operand order: inputs, donated zero outputs, partition ids (last, so
    # the hook's operand_ids[:-1] parameter-order check sees params 0..N-1)
    all_names = in_names + out_names
    if partition_name is not None:
        all_names = all_names + [partition_name]
    donate = tuple(range(n_params, n_params + n_out))

    def _body(*args):
        outs = bass2jax._bass_exec_p.bind(
            *args,
            out_avals=tuple(out_avals),
            in_names=tuple(all_names),
            out_names=tuple(out_names),
            lowering_input_output_aliases=(),
            sim_require_finite=True,
            sim_require_nnan=True,
            nc=nc,
        )
        return tuple(outs)

    devices = jax.devices()[:NCORES]
    mesh = Mesh(np.asarray(devices), ("core",))
    spec = PartitionSpec("core")
    n_extra = 1 if partition_name is not None else 0
    fn = jax.jit(
        shard_map(_body, mesh=mesh,
                  in_specs=(spec,) * (n_params + n_out + n_extra),
                  out_specs=(spec,) * n_out, check_rep=False),
        donate_argnums=donate, keep_unused=True,
    )
    sharding = NamedSharding(mesh, spec)
    concat_in = [
        jax.device_put(
            np.concatenate([np.asarray(in_maps[c][nm]) for c in range(NCORES)],
                           axis=0), sharding)
        for nm in in_names
    ]
    if partition_name is not None:
        pid = jax.device_put(
            np.arange(NCORES, dtype=np.uint32).reshape(NCORES, 1), sharding)
        fn_inner = fn
        fn = lambda *args: fn_inner(*args, pid)
    return fn, concat_in, [(s, sharding) for s in zero_shapes]



# revision 24
# speedup vs baseline: 1.3147x; 1.3147x over previous
"""Causal multi-head attention (B=2, S=2048, D=1024, H=16, DH=64) on 8 TRN2 cores.

Sharding: core c handles batch b = c//4 and head group g = c%4 (4 heads, 256
feature cols).  Each core computes Q/K/V projections for its heads, causal
attention, and a partial output projection; the host sums the 4 partials per
batch.

Device program (per core), all SBUF operands bf16 (PSUM accum stays f32):
  - host pre-transposes X[b] -> XT [D, S] and weight slices, casting to bf16
    (bf16 keeps matmuls at 1 cycle/col at any width; fp8 would breach the
    2e-2 error gate since quantization noise does not average down in
    random-sign contractions).
  - Q^T, K^T kept as [o, s] (2 heads per 128-partition tile); PV stationary
    blocks per (s-chunk, pair): head A = [V | ones] (65 cols; acc rows 0:63
    = out^T, row 64 = denominator), head B = [ones | 0 | V] (128 cols; acc
    row 0 = denominator, rows 64:127 = out^T) so head B's output lands at
    partitions 64-127 directly -- no partition-shift DMA.
  - logits computed transposed (L^T = K_h Q_h^T) into PSUM f32; exp on ACT
    writes bf16 pt tiles; causal triangle masking via gpsimd affine_select
    in place (only the [128,128] diagonal sub-blocks, on the otherwise-idle
    Pool engine); PV accumulates in PSUM f32.
  - causal fine-grain: for diagonal k-chunks only q-cols >= 128j-512t are
    computed in logits/exp/PV.
  - normalization: 1/denominator = exp(-ln d) on ACT (same exp table set),
    NOT DVE reciprocal -- the DVE iterative divide measured ~2-3us per
    [1,512] on HW and head-of-line-blocks the projection copies behind it
    (counterfactual timing: -36us).  ACT ln of each denominator row, PE
    outer-product broadcast of ln d to both partition halves, one ACT
    exp(scale=-1) -> bcast, two DVE muls write outTn in place.  Pair-1
    norms deferred into the next tile's stream (carry).
  - emission interleave: a minimal projection prologue (K/Q pair 0 window 0,
    V s-chunks 0-3), then ALL remaining projection + output-projection work
    is chopped into ~1-2us PE "filler" units drained between each attention
    jg's logits and PV, so PE always has dep-ready work while ACT runs the
    exp stream (phase-alternation costed ~60us of engine idling on HW).
  - PSUM budget (8 banks) caps pipeline depth: lt [128,1024]x2 (4 banks) +
    shared 1-bank pool x4 (proj/out-proj outs, PV accs); norm bcast uses an
    lt slot.

Measured (8 axon-tunneled TRN2 cores): rel err 3.59e-03 vs f32 reference;
~215.5 us/exec by 1-vs-101 hardware-loop slope timing (session-start
baseline 245.4 us measured the same way; CoreSim cost model ~150 us --
the model misses DVE-divide cost, cross-engine sem latency, and HAM).
"""

import os
import numpy as np

# timing-experiment switches (comma-separated): "norecip", "nomask"
_KVAR = set(filter(None, os.environ.get("KVAR", "").split(",")))

B, S, D = 2, 2048, 1024
H, DH = 16, 64
NCORES = 8
GROUPS = 4          # head groups (one per core within a batch)
HPC = H // GROUPS   # heads per core = 4
O = HPC * DH        # per-core feature cols = 256
DC = D // 128       # contraction chunks = 8
NQT = S // 512      # q tiles = 4
NST = S // 128      # s chunks = 16

_PROGRAM = None
LAST_RESULTS = None  # stashed BassKernelResults for test harness introspection


def _build_program(loop_n=1):
    import concourse.bass as bass
    import concourse.tile as tile
    from concourse import bacc, mybir
    from contextlib import ExitStack

    f32 = mybir.dt.float32
    bf16 = mybir.dt.bfloat16
    ts = bass.ts
    Exp = mybir.ActivationFunctionType.Exp
    Ln = mybir.ActivationFunctionType.Ln
    Copy = mybir.ActivationFunctionType.Copy

    nc = bacc.Bacc("TRN2", target_bir_lowering=False, debug=False,
                   num_devices=NCORES)

    xt = nc.dram_tensor("xt", [D, S], bf16, kind="ExternalInput").ap()
    wqt = nc.dram_tensor("wqt", [D, O], bf16, kind="ExternalInput").ap()
    wkt = nc.dram_tensor("wkt", [D, O], bf16, kind="ExternalInput").ap()
    wvt = nc.dram_tensor("wvt", [D, O], bf16, kind="ExternalInput").ap()
    wot = nc.dram_tensor("wot", [O, D], bf16, kind="ExternalInput").ap()
    y = nc.dram_tensor("y", [S, D], f32, kind="ExternalOutput").ap()

    with tile.TileContext(nc) as tc, ExitStack() as ctx:
        per = ctx.enter_context(tc.tile_pool(name="per", bufs=1))
        xtp = ctx.enter_context(tc.tile_pool(name="xtp", bufs=2))
        work = ctx.enter_context(tc.tile_pool(name="work", bufs=6))
        ps_sm = ctx.enter_context(tc.tile_pool(name="ps_sm", bufs=4, space="PSUM"))
        ps_lt = ctx.enter_context(tc.tile_pool(name="ps_lt", bufs=2, space="PSUM"))

        # ---- persistent tiles -------------------------------------------------
        wq_sb = per.tile([128, DC, O], bf16, tag="wq")
        wk_sb = per.tile([128, DC, O], bf16, tag="wk")
        wv_sb = per.tile([128, DC, O], bf16, tag="wv")
        wo_sb = per.tile([128, 2, D], bf16, tag="wo")
        qT = per.tile([128, 2, S], bf16, tag="qT")
        kT = per.tile([128, 2, S], bf16, tag="kT")
        # PV stationary blocks, one per (s-chunk, pair, h01), each 129 cols:
        #   h01=0 (head A): V at cols 0:64, ones at col 64
        #     -> acc rows 0:63 = out_A^T, row 64 = denom_A
        #   h01=1 (head B): ones at col 0, zeros 1:64, V at cols 64:128
        #     -> acc row 0 = denom_B, rows 64:127 = out_B^T (no partition
        #        shift needed; outTn written in place by the norm muls)
        vones = per.tile([128, NST, 2, 2, DH * 2 + 1], bf16, tag="vones")
        outTn = per.tile([128, 2, S], bf16, tag="outTn")

        SH = S // 2

        # ones rows for the outer-product broadcast of softmax denominators:
        # row 64 (head A, out rows 0:63), row 0 (head B, out rows 64:127).
        # f32 because the broadcast rhs (ln of the denominator) is f32.
        ones_bc = per.tile([128, DH], f32, tag="ones_bc")
        nc.vector.memset(ones_bc[DH:DH + 1, :], 1.0)
        nc.vector.memset(ones_bc[0:1, :], 1.0)
        for st in range(NST):
            nc.vector.memset(vones[:, st, :, 0, DH:DH + 1], 1.0)
            nc.vector.memset(vones[:, st, :, 1, 0:DH], 0.0)
            nc.vector.memset(vones[:, st, :, 1, 0:1], 1.0)
        # dummy exp so the ACT exp-table load lands during the input-DMA
        # phase instead of stalling the first attention tile
        warm = per.tile([1, 8], bf16, tag="warm")
        nc.scalar.activation(warm[0:1, :], ones_bc[0:1, 0:8], Exp)

        # ---- phase 0 helper: input DMAs, chunked so the first projection
        # matmuls only wait on their own contraction chunk ---------------------
        def load_inputs():
            wkr = wkt.rearrange("(c p) o -> p c o", p=128)
            wqr = wqt.rearrange("(c p) o -> p c o", p=128)
            wvr = wvt.rearrange("(c p) o -> p c o", p=128)
            xt_sbs = [xtp.tile([128, DC, SH], bf16, tag="xt", name="xt")
                      for _ in range(2)]
            for dc in range(DC):
                nc.sync.dma_start(wk_sb[:, dc, :], wkr[:, dc, :])
                nc.sync.dma_start(
                    xt_sbs[0][:, dc, :], xt[dc * 128:(dc + 1) * 128, 0:SH])
            for dc in range(DC):
                nc.sync.dma_start(wq_sb[:, dc, :], wqr[:, dc, :])
                nc.sync.dma_start(wv_sb[:, dc, :], wvr[:, dc, :])
            for dc in range(DC):
                nc.sync.dma_start(
                    xt_sbs[1][:, dc, :], xt[dc * 128:(dc + 1) * 128, SH:S])
            nc.sync.dma_start(wo_sb[:], wot.rearrange("(c p) m -> p c m", p=128))
            return xt_sbs

        # (wk and wq stream first so the prologue's K/Q matmuls -- and with
        # them the first logits feeding ACT -- start as early as possible;
        # the V projections are fillers inside attention tile 0 and wv lands
        # in time for them)

        # ---- PE filler units (~0.85-1.7us each): projections and output
        # projection chunks, interleaved between attention jg iterations so
        # PE always has dep-ready work while the ACT exp stream runs ----------
        def kq_unit(xt_sbs, sh, w_sb, dst, pt_i, qt):
            def run():
                xt_sb = xt_sbs[sh]
                ps = ps_sm.tile([128, 512], f32, tag="sm", name="mm")
                for dc in range(DC):
                    nc.tensor.matmul(
                        ps[:],
                        w_sb[:, dc, ts(pt_i, 128)],
                        xt_sb[:, dc, ts(qt, 512)],
                        start=(dc == 0), stop=(dc == DC - 1),
                    )
                # psum f32 -> bf16 persistent on DVE (ACT is the attention
                # bottleneck engine; DVE has slack)
                nc.vector.tensor_copy(
                    dst[:, pt_i, ts(sh * 2 + qt, 512)], ps[:])
            return run

        def v_unit(xt_sbs, sh, st_l):
            def run():
                xt_sb = xt_sbs[sh]
                st = sh * (SH // 128) + st_l
                ps = ps_sm.tile([128, O], f32, tag="sm", name="mm")
                for dc in range(DC):
                    nc.tensor.matmul(
                        ps[:],
                        xt_sb[:, dc, ts(st_l, 128)],
                        wv_sb[:, dc, :],
                        start=(dc == 0), stop=(dc == DC - 1),
                    )
                # ps cols are (pair, h01, d); A-heads to block 0 cols 0:64,
                # B-heads to block 1 cols 64:128
                psr = ps[:].rearrange("p (pr hd d) -> p pr hd d", pr=2, hd=2)
                nc.vector.tensor_copy(
                    vones[:, st, :, 0, 0:DH], psr[:, :, 0, :])
                nc.vector.tensor_copy(
                    vones[:, st, :, 1, DH:2 * DH], psr[:, :, 1, :])
            return run

        def op_unit(st):
            def run():
                ys = work.tile([128, 1024], f32, tag="ystage")
                for mt in range(2):
                    ps = ps_sm.tile([128, 512], f32, tag="sm", name="mm")
                    for pair in range(2):
                        nc.tensor.matmul(
                            ps[:],
                            outTn[:, pair, ts(st, 128)],
                            wo_sb[:, pair, ts(mt, 512)],
                            start=(pair == 0), stop=(pair == 1),
                        )
                    nc.vector.tensor_copy(ys[:, ts(mt, 512)], ps[:])
                nc.sync.dma_start(y[ts(st, 128), :], ys[:])
            return run

        # ---- attention for one q tile; `fillers` (projection / out-proj
        # units) are drained between each jg's logits and PV so PE stays busy
        # while that jg's exps run on ACT -------------------------------------
        def attn_tile(t, fillers=(), carry_in=None):
            fillers = list(fillers)
            nfill = len(fillers)
            total_slots = 2 * (2 * t + 2)
            slot = drained = 0

            def drain():
                nonlocal slot, drained
                slot += 1
                while drained < nfill * slot // total_slots:
                    fillers[drained]()
                    drained += 1

            carry = carry_in
            for pair in range(2):
                accs = [ps_sm.tile([DH + 1, 512], f32, tag="sm", name="acc"),
                        ps_sm.tile([128, 512], f32, tag="sm", name="acc")]
                njp = 2 * t + 2  # j-pairs covering k chunks 0..4t+3
                for jg in range(njp):
                    if jg == 1 and pair == 0 and carry is not None:
                        carry()  # deferred previous-pair normalization
                        carry = None
                    j0, j1 = 2 * jg, 2 * jg + 1
                    # fine-grain causal: for diagonal k-chunks only q-cols
                    # >= 128j - 512t participate
                    c0 = max(0, 128 * j0 - 512 * t)
                    c1 = max(0, 128 * j1 - 512 * t)
                    lts = [ps_lt.tile([128, 1024], f32, tag="lt", name="lt")
                           for _ in range(2)]
                    # j1's logits start at c0 too (extra strip fully
                    # masked below) so exp runs as ONE ACT instruction
                    for j_half, j, c in ((0, j0, c0), (1, j1, c0)):
                        for h01 in range(2):
                            bp = 64 * h01
                            nc.tensor.matmul(
                                lts[h01][:, j_half * 512 + c:(j_half + 1) * 512],
                                kT[bp:bp + 64, pair, ts(j, 128)],
                                qT[bp:bp + 64, pair, 512 * t + c:512 * (t + 1)],
                                start=True, stop=True,
                            )
                    drain()  # PE filler between this jg's logits and its PV
                    for h01 in range(2):
                        pt = work.tile([128, 1024], bf16, tag="pt", bufs=6)
                        if "halfexp" in _KVAR:  # timing counterfactual
                            nc.scalar.activation(pt[:, 0:512], lts[h01][:, 0:512],
                                                 Exp, scale=DH ** -0.5)
                        elif c0 == 0:
                            nc.scalar.activation(pt[:], lts[h01][:], Exp,
                                                 scale=DH ** -0.5)
                        else:
                            nc.scalar.activation(
                                pt[:, c0:512], lts[h01][:, c0:512], Exp,
                                scale=DH ** -0.5)
                            nc.scalar.activation(
                                pt[:, 512 + c0:1024], lts[h01][:, 512 + c0:1024],
                                Exp, scale=DH ** -0.5)
                        # zero the strictly-upper triangle of diagonal blocks
                        # (j1 region widened to cover the masked [c0,c1) strip)
                        for j_half, j, c in ((0, j0, c0), (1, j1, c1)):
                            if "nomask" in _KVAR:  # timing counterfactual
                                continue
                            if j >= 4 * t:
                                lo = j_half * 512 + (c0 if j_half else c)
                                hi = j_half * 512 + c + 128
                                nc.gpsimd.affine_select(
                                    out=pt[:, lo:hi],
                                    in_=pt[:, lo:hi],
                                    compare_op=mybir.AluOpType.is_ge,
                                    fill=0.0,
                                    base=lo - (j_half * 512 + c),
                                    pattern=[[1, hi - lo]],
                                    channel_multiplier=-1,
                                )
                        lhsT = (vones[:, :, pair, 0, 0:DH + 1] if h01 == 0
                                else vones[:, :, pair, 1, 0:2 * DH])
                        for j_half, j, c in ((0, j0, c0), (1, j1, c1)):
                            nc.tensor.matmul(
                                accs[h01][:, c:512],
                                lhsT[:, j, :],
                                pt[:, j_half * 512 + c:(j_half + 1) * 512],
                                start=(jg == 0 and j_half == 0),
                                stop=(jg == njp - 1 and j_half == 1),
                            )
                def norm(accs=accs, pair=pair):
                    # softmax 1/denominator WITHOUT the DVE iterative-divide
                    # reciprocal (measured ~2-3us per [1,512] on HW): ACT ln
                    # of each denominator row (A: row 64 of acc0, B: row 0 of
                    # acc1), PE outer-product broadcast of ln(d) to the two
                    # partition halves, then one ACT exp(-x) writes the
                    # broadcast reciprocal; two DVE muls write outTn in place
                    lnd = work.tile([128, 512], f32, tag="recip")
                    nc.scalar.activation(lnd[DH:DH + 1, :],
                                         accs[0][DH:DH + 1, :], Ln)
                    nc.scalar.activation(lnd[0:1, :],
                                         accs[1][0:1, :], Ln)
                    bc_ps = ps_lt.tile([128, 1024], f32, tag="lt", name="bcps")
                    nc.tensor.matmul(bc_ps[0:DH, 0:512],
                                     ones_bc[DH:DH + 1, :],
                                     lnd[DH:DH + 1, :],
                                     start=True, stop=True)
                    nc.tensor.matmul(bc_ps[DH:128, 0:512],
                                     ones_bc[0:1, :],
                                     lnd[0:1, :],
                                     start=True, stop=True)
                    bcast = work.tile([128, 512], bf16, tag="bcast")
                    with nc.allow_low_precision(
                            reason="softmax reciprocal feeds bf16 pipeline"):
                        nc.scalar.activation(bcast[:, :], bc_ps[:, 0:512],
                                             Exp, scale=-1.0)
                    nc.vector.tensor_mul(outTn[0:DH, pair, ts(t, 512)],
                                         accs[0][0:DH, :], bcast[0:DH, :])
                    nc.vector.tensor_mul(outTn[DH:128, pair, ts(t, 512)],
                                         accs[1][DH:128, :], bcast[DH:128, :])
                if pair == 0:
                    norm()
                else:
                    carry = norm
            while drained < nfill:  # leftovers (shouldn't normally trigger)
                fillers[drained]()
                drained += 1
            return carry

        # ---- emission: a short projection prologue covering only attention
        # tile 0's needs, then attention tiles with all remaining projection
        # and output-projection work drained as per-jg PE fillers -------------
        def body():
            xt_sbs = load_inputs()
            # prologue: just K/Q head-pair 0 for q-window 0 -- the first
            # logits (and with them the ACT exp stream) start ~3.4us sooner;
            # V s-chunks ride as fillers and are ready before their PVs
            kq_unit(xt_sbs, 0, wk_sb, kT, 0, 0)()
            kq_unit(xt_sbs, 0, wq_sb, qT, 0, 0)()
            c = attn_tile(0, fillers=[
                v_unit(xt_sbs, 0, 0),
                v_unit(xt_sbs, 0, 1),
                v_unit(xt_sbs, 0, 2),
                v_unit(xt_sbs, 0, 3),
                kq_unit(xt_sbs, 0, wk_sb, kT, 1, 0),
                kq_unit(xt_sbs, 0, wq_sb, qT, 1, 0),
                kq_unit(xt_sbs, 0, wk_sb, kT, 0, 1),
                kq_unit(xt_sbs, 0, wk_sb, kT, 1, 1),
                kq_unit(xt_sbs, 0, wq_sb, qT, 0, 1),
                kq_unit(xt_sbs, 0, wq_sb, qT, 1, 1),
                v_unit(xt_sbs, 0, 4),
                v_unit(xt_sbs, 0, 5),
            ])
            c = attn_tile(1, fillers=[
                v_unit(xt_sbs, 0, 6),
                v_unit(xt_sbs, 0, 7),
                kq_unit(xt_sbs, 1, wk_sb, kT, 0, 0),
                kq_unit(xt_sbs, 1, wk_sb, kT, 1, 0),
                kq_unit(xt_sbs, 1, wq_sb, qT, 0, 0),
                kq_unit(xt_sbs, 1, wq_sb, qT, 1, 0),
                op_unit(0), op_unit(1), op_unit(2), op_unit(3),
            ], carry_in=c)
            c = attn_tile(2, fillers=[
                v_unit(xt_sbs, 1, 0),
                v_unit(xt_sbs, 1, 1),
                v_unit(xt_sbs, 1, 2),
                v_unit(xt_sbs, 1, 3),
                v_unit(xt_sbs, 1, 4),
                v_unit(xt_sbs, 1, 5),
                kq_unit(xt_sbs, 1, wk_sb, kT, 0, 1),
                kq_unit(xt_sbs, 1, wk_sb, kT, 1, 1),
                kq_unit(xt_sbs, 1, wq_sb, qT, 0, 1),
                kq_unit(xt_sbs, 1, wq_sb, qT, 1, 1),
                op_unit(4), op_unit(5), op_unit(6), op_unit(7),
            ], carry_in=c)
            c = attn_tile(3, fillers=[
                v_unit(xt_sbs, 1, 6),
                v_unit(xt_sbs, 1, 7),
                op_unit(8), op_unit(9), op_unit(10), op_unit(11),
            ], carry_in=c)
            c()
            for st in range(12, 16):
                op_unit(st)()

        if loop_n == 1:
            body()
        else:
            with tc.For_i(0, loop_n, 1):
                body()

    nc.compile()
    return nc


def _get_program(loop_n=1):
    global _PROGRAM
    if _PROGRAM is None:
        _PROGRAM = {}
    if loop_n not in _PROGRAM:
        _PROGRAM[loop_n] = _build_program(loop_n)
    return _PROGRAM[loop_n]


def kernel(X, Wq, Wk, Wv, Wo):
    global LAST_RESULTS
    from concourse.bass_utils import run_bass_kernel_spmd

    X = np.asarray(X, dtype=np.float32)
    Wq = np.asarray(Wq, dtype=np.float32)
    Wk = np.asarray(Wk, dtype=np.float32)
    Wv = np.asarray(Wv, dtype=np.float32)
    Wo = np.asarray(Wo, dtype=np.float32)

    nc = _get_program()
    in_maps = _make_in_maps(X, Wq, Wk, Wv, Wo)
    res = run_bass_kernel_spmd(nc, in_maps, list(range(NCORES)))
    LAST_RESULTS = res

    out = np.empty((B, S, D), dtype=np.float32)
    for b in range(B):
        acc = res.results[b * GROUPS]["y"].astype(np.float32)
        for g in range(1, GROUPS):
            acc = acc + res.results[b * GROUPS + g]["y"]
        out[b] = acc
    return out


def _make_in_maps(X, Wq, Wk, Wv, Wo):
    import ml_dtypes
    bf16 = ml_dtypes.bfloat16
    xts = [np.ascontiguousarray(X[b].T).astype(bf16) for b in range(B)]
    in_maps = []
    for c in range(NCORES):
        b, g = divmod(c, GROUPS)
        rows = slice(g * O, (g + 1) * O)
        in_maps.append({
            "xt": xts[b],
            "wqt": np.ascontiguousarray(Wq[rows, :].T).astype(bf16),
            "wkt": np.ascontiguousarray(Wk[rows, :].T).astype(bf16),
            "wvt": np.ascontiguousarray(Wv[rows, :].T).astype(bf16),
            "wot": np.ascontiguousarray(Wo[:, rows].T).astype(bf16),
        })
    return in_maps


def build_timed_callable(in_maps=None, loop_n=1):
    """Build the same sharded jit callable bass2jax uses, with inputs
    pre-placed on the 8 devices, for repeat-timing the NEFF execution.

    With loop_n=K the device program wraps the whole kernel body (including
    input DMAs) in a K-iteration hardware loop, so per-exec device time can
    be measured as a slope between two loop counts, cancelling the (large,
    noisy) axon dispatch overhead."""
    import jax
    import numpy as np
    from jax.sharding import Mesh, PartitionSpec, NamedSharding
    from jax.experimental.shard_map import shard_map
    from concourse import bass2jax, mybir

    nc = _get_program(loop_n)
    bass2jax.install_neuronx_cc_hook()

    if in_maps is None:
        import test as _t
        inputs, _ = _t.get_reference_data()
        in_maps = _make_in_maps(**inputs)

    partition_name = (
        nc.partition_id_tensor.name if nc.partition_id_tensor else None)
    in_names, out_names, out_avals, zero_shapes = [], [], [], []
    for alloc in nc.m.functions[0].allocations:
        if not isinstance(alloc, mybir.MemoryLocationSet):
            continue
        name = alloc.memorylocations[0].name
        if alloc.kind == "ExternalInput":
            if name != partition_name:
                in_names.append(name)
        elif alloc.kind == "ExternalOutput":
            out_names.append(name)
            shape = tuple(alloc.tensor_shape)
            out_avals.append(
                jax.core.ShapedArray(shape, mybir.dt.np(alloc.dtype)))
            zero_shapes.append((NCORES * shape[0], *shape[1:]))
    n_params = len(in_names)
    n_out = len(out_names)
    # operand order: inputs, donated zero outputs, partition ids (last, so
    # the hook's operand_ids[:-1] parameter-order check sees params 0..N-1)
    all_names = in_names + out_names
    if partition_name is not None:
        all_names = all_names + [partition_name]
    donate = tuple(range(n_params, n_params + n_out))

    def _body(*args):
        outs = bass2jax._bass_exec_p.bind(
            *args,
            out_avals=tuple(out_avals),
            in_names=tuple(all_names),
            out_names=tuple(out_names),
            lowering_input_output_aliases=(),
            sim_require_finite=True,
            sim_require_nnan=True,
            nc=nc,
        )
        return tuple(outs)

    devices = jax.devices()[:NCORES]
    mesh = Mesh(np.asarray(devices), ("core",))
    spec = PartitionSpec("core")
    n_extra = 1 if partition_name is not None else 0
    fn = jax.jit(
        shard_map(_body, mesh=mesh,
                  in_specs=(spec,) * (n_params + n_out + n_extra),
                  out_specs=(spec,) * n_out, check_rep=False),
        donate_argnums=donate, keep_unused=True,
    )
    sharding = NamedSharding(mesh, spec)
    concat_in = [
        jax.device_put(
            np.concatenate([np.asarray(in_maps[c][nm]) for c in range(NCORES)],
                           axis=0), sharding)
        for nm in in_names
    ]
    if partition_name is not None:
        pid = jax.device_put(
            np.arange(NCORES, dtype=np.uint32).reshape(NCORES, 1), sharding)
        fn_inner = fn
        fn = lambda *args: fn_inner(*args, pid)
    return fn, concat_in, [(s, sharding) for s in zero_shapes]

